# revision 3
# baseline (speedup 1.0000x reference)
"""Fused conv1x1-attention-FFN kernel for 8 trn2 NeuronCores.

Reference computation (per batch b of 4, N = 64*64 = 4096 pixels, C = 256):
    q = Wq @ x_q + bq ; k = Wk @ x_kv + bk ; v = Wv @ x_kv + bv      [C, N]
    attn = softmax_over_keys(q^T k)                                   [N, N]
    av = v @ attn^T                                                   [C, N]
    out = W2 @ relu(W1 @ av + b1) + b2                                [C, N]

Sharding: 8 cores = 4 batches x 2 query-row halves. Each core computes the
full K/V for its batch (cheap: 0.5 GMAC vs 4.3 GMAC attention) and attends
its 2048 query rows against all 4096 keys — no collectives needed.

On-chip layout (all matmuls contract over the partition dim):
    scores are computed TRANSPOSED: S^T[m, n] = sum_c k[c,m] q[c,n] so that
    the attention-value product av[c,n] = sum_m v^T[m,c] E[m,n] needs no
    on-chip transpose; v is projected directly into v^T[m,c] layout by using
    x_kv as the stationary operand. Softmax denominators come from a
    ones-column matmul over E; normalization is folded in after av via a
    broadcast matmul of the reciprocal row.

Compute dtype: float32r (TF32-like, ~1.5e-4 matmul error, full PE speed at
free-dim >= 256). PSUM accumulation is fp32.
"""
import sys

sys.path.insert(0, "/opt/trn_rl_repo")

import numpy as np
from concourse import bass, bacc, mybir, tile
from concourse.bass_utils import run_bass_kernel_spmd

F32 = mybir.dt.float32
CDT = mybir.dt.float32r  # compute dtype for PE operands

B, C, H, W = 4, 256, 64, 64
N = H * W              # 4096 keys per batch
NL = N // 2            # 2048 query rows per core
CT = C // 128          # 2 channel tiles
MT = N // 128          # 32 key tiles
NCH = 512              # query-column chunk
NJ = NL // NCH         # 4 chunks
AF = mybir.ActivationFunctionType


def _build():
    nc = bacc.Bacc(None, target_bir_lowering=False, debug=False)

    xq_d = nc.declare_dram_parameter("xq", [128, CT, NL], F32, isOutput=False)
    xkv_d = nc.declare_dram_parameter("xkv", [128, CT, N], F32, isOutput=False)
    w_d = {}
    for name in ("wqt", "wkt", "wvt", "w1t", "w2t"):
        w_d[name] = nc.declare_dram_parameter(name, [128, CT, C], F32, isOutput=False)
    b_d = {}
    for name in ("bq", "bk", "b1", "b2"):
        b_d[name] = nc.declare_dram_parameter(name, [128, CT, 1], F32, isOutput=False)
    bv_d = nc.declare_dram_parameter("bvb", [128, C], F32, isOutput=False)
    out_d = nc.declare_dram_parameter("out", [128, CT, NL], F32, isOutput=True)

    with tile.TileContext(nc) as tc:
        with (
            tc.tile_pool(name="const", bufs=1) as cpool,
            tc.tile_pool(name="stage", bufs=2) as spool,
            tc.tile_pool(name="big", bufs=1) as bpool,
            tc.tile_pool(name="work", bufs=2) as wpool,
            tc.tile_pool(name="et", bufs=3) as epool,
            tc.tile_pool(name="psum", bufs=1, space="PSUM") as pp,
            tc.tile_pool(name="psum2", bufs=2, space="PSUM") as pp2,
        ):
            # ---- constants: weights (rounded to CDT), biases ----
            w_r = {}
            for name in ("wqt", "wkt", "wvt", "w1t", "w2t"):
                ws = spool.tile([128, CT, C], F32, tag="wstage")
                nc.sync.dma_start(ws[:], w_d[name][:])
                w_r[name] = cpool.tile([128, CT, C], CDT, tag=f"{name}_r", name=f"{name}_r")
                nc.vector.tensor_copy(w_r[name][:], ws[:])
            b_s = {}
            for name in ("bq", "bk", "b1", "b2"):
                b_s[name] = cpool.tile([128, CT, 1], F32, tag=f"{name}_s", name=f"{name}_s")
                nc.sync.dma_start(b_s[name][:], b_d[name][:])
            bv_s = cpool.tile([128, C], F32, tag="bv_s")
            nc.sync.dma_start(bv_s[:], bv_d[:])
            ones_f = cpool.tile([128, 1], F32, tag="ones_f")
            nc.vector.memset(ones_f[:], 1.0)
            ones_r = cpool.tile([128, 1], CDT, tag="ones_r")
            nc.vector.tensor_copy(ones_r[:], ones_f[:])
            onesrow = cpool.tile([1, 128], F32, tag="onesrow")
            nc.vector.memset(onesrow[:], 1.0)

            # ---- load + round inputs (streamed; stage slots shared) ----
            xq_r = bpool.tile([128, CT, NL], CDT, tag="xq_r")
            for mc in range(2):
                st = spool.tile([128, CT, 1024], F32, tag="xstage")
                nc.sync.dma_start(st[:], xq_d[:, :, mc * 1024:(mc + 1) * 1024])
                nc.vector.tensor_copy(xq_r[:, :, mc * 1024:(mc + 1) * 1024], st[:])
            xkv_r = bpool.tile([128, CT, N], CDT, tag="xkv_r")
            for mc in range(4):
                st = spool.tile([128, CT, 1024], F32, tag="xstage")
                nc.sync.dma_start(st[:], xkv_d[:, :, mc * 1024:(mc + 1) * 1024])
                nc.vector.tensor_copy(xkv_r[:, :, mc * 1024:(mc + 1) * 1024], st[:])

            # ---- projections ----
            # q[c, n] / k[c, m]: out channel tile on partitions
            q_r = bpool.tile([128, CT, NL], CDT, tag="q_r")
            for ct in range(CT):
                for j in range(NJ):
                    ps = pp2.tile([128, NCH], F32, tag="st")
                    sl = slice(j * NCH, (j + 1) * NCH)
                    for ci in range(CT):
                        nc.tensor.matmul(
                            ps[:], w_r["wqt"][:, ci, ct * 128:(ct + 1) * 128],
                            xq_r[:, ci, sl], start=(ci == 0), stop=(ci == CT - 1))
                    nc.vector.tensor_scalar_add(q_r[:, ct, sl], ps[:], b_s["bq"][:, ct, :])
            k_r = bpool.tile([128, CT, N], CDT, tag="k_r")
            for ct in range(CT):
                for j in range(N // NCH):
                    ps = pp2.tile([128, NCH], F32, tag="st")
                    sl = slice(j * NCH, (j + 1) * NCH)
                    for ci in range(CT):
                        nc.tensor.matmul(
                            ps[:], w_r["wkt"][:, ci, ct * 128:(ct + 1) * 128],
                            xkv_r[:, ci, sl], start=(ci == 0), stop=(ci == CT - 1))
                    nc.vector.tensor_scalar_add(k_r[:, ct, sl], ps[:], b_s["bk"][:, ct, :])
            # v^T[m, c]: key tile on partitions (x_kv is the stationary operand)
            vt_r = bpool.tile([128, MT, C], CDT, tag="vt_r")
            for mi in range(MT):
                ps = pp2.tile([128, C], F32, tag="st")
                for ci in range(CT):
                    nc.tensor.matmul(
                        ps[:], xkv_r[:, ci, mi * 128:(mi + 1) * 128],
                        w_r["wvt"][:, ci, :], start=(ci == 0), stop=(ci == CT - 1))
                nc.vector.tensor_add(vt_r[:, mi, :], ps[:], bv_s[:])

            # ---- attention + FFN, per 512-wide query chunk ----
            for j in range(NJ):
                sl = slice(j * NCH, (j + 1) * NCH)
                av0 = pp.tile([128, NCH], F32, tag="av0")
                av1 = pp.tile([128, NCH], F32, tag="av1")
                smp = pp.tile([1, NCH], F32, tag="sum")
                for mi in range(MT):
                    sp = pp2.tile([128, NCH], F32, tag="st")
                    for ci in range(CT):
                        nc.tensor.matmul(
                            sp[:], k_r[:, ci, mi * 128:(mi + 1) * 128],
                            q_r[:, ci, sl], start=(ci == 0), stop=(ci == CT - 1))
                    et = epool.tile([128, NCH], CDT, tag="et")
                    nc.scalar.activation(et[:], sp[:], AF.Exp)
                    first, last = mi == 0, mi == MT - 1
                    nc.tensor.matmul(smp[:], ones_r[:], et[:], start=first, stop=last)
                    nc.tensor.matmul(av0[:], vt_r[:, mi, 0:128], et[:],
                                     start=first, stop=last)
                    nc.tensor.matmul(av1[:], vt_r[:, mi, 128:256], et[:],
                                     start=first, stop=last)
                # normalize: av * (1/rowsum) broadcast across partitions
                recip = wpool.tile([1, NCH], F32, tag="recip")
                nc.vector.reciprocal(recip[:], smp[:])
                rbp = pp2.tile([128, NCH], F32, tag="ffn")
                nc.tensor.matmul(rbp[:], onesrow[:], recip[:], start=True, stop=True)
                rb = wpool.tile([128, NCH], F32, tag="rb")
                nc.vector.tensor_copy(rb[:], rbp[:])
                avn = wpool.tile([128, CT, NCH], CDT, tag="avn")
                nc.vector.tensor_mul(avn[:, 0, :], av0[:], rb[:])
                nc.vector.tensor_mul(avn[:, 1, :], av1[:], rb[:])
                # FFN
                hid = wpool.tile([128, CT, NCH], CDT, tag="hid")
                for ot in range(CT):
                    hp = pp2.tile([128, NCH], F32, tag="ffn")
                    for ci in range(CT):
                        nc.tensor.matmul(
                            hp[:], w_r["w1t"][:, ci, ot * 128:(ot + 1) * 128],
                            avn[:, ci, :], start=(ci == 0), stop=(ci == CT - 1))
                    nc.scalar.activation(hid[:, ot, :], hp[:], AF.Relu,
                                         bias=b_s["b1"][:, ot, :])
                outp = wpool.tile([128, CT, NCH], F32, tag="outp")
                for ot in range(CT):
                    op = pp2.tile([128, NCH], F32, tag="ffn")
                    for ci in range(CT):
                        nc.tensor.matmul(
                            op[:], w_r["w2t"][:, ci, ot * 128:(ot + 1) * 128],
                            hid[:, ci, :], start=(ci == 0), stop=(ci == CT - 1))
                    nc.vector.tensor_scalar_add(outp[:, ot, :], op[:],
                                                b_s["b2"][:, ot, :])
                nc.sync.dma_start(out_d[:, :, sl], outp[:])
    nc.compile()
    return nc


_NC_CACHE = None


def _get_nc():
    global _NC_CACHE
    if _NC_CACHE is None:
        _NC_CACHE = _build()
    return _NC_CACHE


def _fold(a):
    """[C, X] -> [128, CT, X] with channel tile as middle dim, contiguous."""
    x = np.ascontiguousarray(np.asarray(a, dtype=np.float32))
    return np.ascontiguousarray(
        x.reshape(CT, 128, -1).transpose(1, 0, 2))


def _make_in_maps(inputs):
    query_input = np.asarray(inputs["query_input"], np.float32).reshape(B, C, N)
    key_value_input = np.asarray(inputs["key_value_input"], np.float32).reshape(B, C, N)
    base = {
        "wqt": _fold(np.asarray(inputs["Wq"], np.float32).T),
        "wkt": _fold(np.asarray(inputs["Wk"], np.float32).T),
        "wvt": _fold(np.asarray(inputs["Wv"], np.float32).T),
        "w1t": _fold(np.asarray(inputs["W1"], np.float32).T),
        "w2t": _fold(np.asarray(inputs["W2"], np.float32).T),
        "bq": _fold(np.asarray(inputs["bq"], np.float32).reshape(C, 1)),
        "bk": _fold(np.asarray(inputs["bk"], np.float32).reshape(C, 1)),
        "b1": _fold(np.asarray(inputs["b1"], np.float32).reshape(C, 1)),
        "b2": _fold(np.asarray(inputs["b2"], np.float32).reshape(C, 1)),
        "bvb": np.ascontiguousarray(
            np.broadcast_to(np.asarray(inputs["bv"], np.float32)[None, :], (128, C))),
    }
    in_maps = []
    for core in range(8):
        b, h = divmod(core, 2)
        m = dict(base)
        m["xq"] = _fold(query_input[b][:, h * NL:(h + 1) * NL])
        m["xkv"] = _fold(key_value_input[b])
        in_maps.append(m)
    return in_maps


def kernel(query_input, key_value_input, Wq, bq, Wk, bk, Wv, bv, W1, b1, W2, b2):
    in_maps = _make_in_maps(dict(
        query_input=query_input, key_value_input=key_value_input,
        Wq=Wq, bq=bq, Wk=Wk, bk=bk, Wv=Wv, bv=bv, W1=W1, b1=b1, W2=W2, b2=b2))
    nc = _get_nc()
    res = run_bass_kernel_spmd(nc, in_maps, core_ids=list(range(8)))

    out = np.empty((B, C, N), dtype=np.float32)
    for core in range(8):
        b, h = divmod(core, 2)
        o = res.results[core]["out"]  # [128, CT, NL]
        out[b][:, h * NL:(h + 1) * NL] = o.transpose(1, 0, 2).reshape(C, NL)
    return out.reshape(B, C, H, W)
